# revision 19
# baseline (speedup 1.0000x reference)
"""BatchTreeEncoder Trainium2 kernel.

Forest of B=1024 identical complete 4-ary trees (341 nodes, 5 levels).
reference: e_v = W emb[tok_v] + b; s_v = subtree sum of e; out = per-tree
elementwise max over all s_v.

Strategy (data-parallel over trees, 128 trees/core on 8 cores, all
engines balanced):
  * Host gathers embeddings bf16 and TRANSPOSED ([128 channels, node
    cols]) in quarter-major (Lambda) order: each level's superchunk
    block is [q0|q1|q2|q3] where q_c holds the c-th children of the
    parent level's own Lambda order (Lambda_0 = tree order).  Every
    4-way child fold -- sum or max, at every level -- is then three
    tensor_tensor ops on CONTIGUOUS slices (guaranteed 2x DVE mode).
  * PE projects leaves and s3 sums with stationary W^T (N=512 streams,
    no transposes, no LDWEIGHTS thrash), and builds upper-level
    projections by PSUM-accumulating identity matmuls over the drained
    child projections on top of W^T @ e_raw.
  * ACT drains every projection PSUM->SBUF bf16 with the node bias
    fused (+b, +5b for s3); the per-drain +b telescopes to exactly
    |subtree|*b for every node value.
  * DVE folds raw leaf sums, the leaf max quartet, and the max cascade
    down the levels; the last fold writes the per-tree answer directly.
  * Final [d,tree]->[tree,d] via HWDGE DMA transpose.

The installed walrus gives every engine instruction a single sync-wait
slot, so _build_nc runs a fixpoint: build, find instructions that were
assigned >1 wait, rebuild with carrier nops (one wait each) glued
immediately before those instructions on the same engine.
"""

import sys

sys.path.insert(0, "/opt/trn_rl_repo")

import numpy as np

B = 1024
NPT = 341
VOCAB = 50000
D = 128
NCORES = 8
TPC = B // NCORES          # 128 trees per core
SC = 4                     # superchunks per core
TPS = TPC // SC            # 32 trees per superchunk

N4 = TPS * 256             # 8192 leaf cols per superchunk
N3 = TPS * 64              # 2048
N2 = TPS * 16              # 512
N1 = TPS * 4               # 128
N0 = TPS * 1               # 32
UW = N3 + N2 + N1 + N0     # 2720 upper cols per superchunk
U3, U2, U1, U0 = 0, N3, N3 + N2, N3 + N2 + N1

_compiled = {}


def _build_once(sites):
    """Build the kernel; emission index i gets sites.get(i, 0) carrier nops
    glued immediately before it on its engine. Returns (nc, name2idx)."""
    import concourse.bass as bass
    import concourse.mybir as mybir
    import concourse.tile as tile
    from bass_rust import add_dep_helper as _adh

    f32 = mybir.dt.float32
    bf16 = mybir.dt.bfloat16
    T = mybir.ActivationFunctionType
    MAX = mybir.AluOpType.max
    ADD = mybir.AluOpType.add

    nc = bass.Bass()
    gxld = nc.declare_dram_parameter("gxl", [128, SC * N4], bf16, isOutput=False)
    gxud = nc.declare_dram_parameter("gxu", [128, SC * UW], bf16, isOutput=False)
    wtd = nc.declare_dram_parameter("wt", [D, D], bf16, isOutput=False)
    idd = nc.declare_dram_parameter("ident", [D, D], bf16, isOutput=False)
    bcd = nc.declare_dram_parameter("bcol", [D, 2], f32, isOutput=False)   # [b, 5b]
    outd = nc.declare_dram_parameter("out", [TPC, D], f32, isOutput=True)

    emidx = [0]
    name2idx = {}
    last_on = {}

    def em(eng, maker):
        # emission wrapper: chains each engine's instructions in emission
        # order (nosync deps only) so carrier nops stay adjacent to the
        # instruction whose excess waits they will carry
        i = emidx[0]
        emidx[0] += 1
        for _ in range(sites.get(i, 0)):
            nop = eng.nop(nofuse=True)
            if last_on.get(id(eng)) is not None:
                _adh(nop.ins, last_on[id(eng)], sync=False, reason="carrier order")
            last_on[id(eng)] = nop.ins
        inst = maker()
        if last_on.get(id(eng)) is not None:
            _adh(inst.ins, last_on[id(eng)], sync=False, reason="carrier order")
        last_on[id(eng)] = inst.ins
        name2idx[inst.ins.name] = i
        return inst

    with tile.TileContext(nc) as tc:
        with (
            tc.tile_pool(name="const", bufs=1) as cpool,
            tc.tile_pool(name="gq", bufs=3) as gqpool,
            tc.tile_pool(name="gu", bufs=3) as gupool,
            tc.tile_pool(name="lp", bufs=3) as lppool,
            tc.tile_pool(name="wk", bufs=2) as wkpool,
            tc.tile_pool(name="dr", bufs=2) as drpool,
            tc.tile_pool(name="psq", bufs=2, space="PSUM") as psq,
        ):
            def pemm(**kw):
                return em(nc.tensor, lambda: nc.tensor.matmul(**kw))

            def aact(out, in_, bias):
                return em(nc.scalar, lambda: nc.scalar.activation(
                    out=out, in_=in_, func=T.Identity, bias=bias, scale=1.0))

            def vtt(op, out, in0, in1):
                return em(nc.vector, lambda: nc.vector.tensor_tensor(
                    out=out, in0=in0, in1=in1, op=op))

            wt = cpool.tile([D, D], bf16)
            em(nc.sync, lambda: nc.sync.dma_start(out=wt[:], in_=wtd[:]))
            ident = cpool.tile([D, D], bf16)
            em(nc.sync, lambda: nc.sync.dma_start(out=ident[:], in_=idd[:]))
            bcol = cpool.tile([D, 2], f32)
            em(nc.sync, lambda: nc.sync.dma_start(out=bcol[:], in_=bcd[:]))
            b1 = bcol[:, 0:1]
            b5 = bcol[:, 1:2]

            R = cpool.tile([D, TPC], bf16)     # per-tree answers, [d', tree]

            for s in range(SC):
                gl = gqpool.tile([128, N4], bf16, tag="gl")
                em(nc.sync, lambda gl=gl, s=s: nc.sync.dma_start(
                    out=gl[:], in_=gxld[:, N4 * s:N4 * (s + 1)]))
                gq = [gl[:, N3 * c:N3 * (c + 1)] for c in range(4)]
                gu = gupool.tile([128, UW], bf16, tag="gu")
                em(nc.sync, lambda gu=gu, s=s: nc.sync.dma_start(
                    out=gu[:], in_=gxud[:, UW * s:UW * (s + 1)]))

                # ---- leaf projections + biased drains (node values) ----
                Lp = lppool.tile([128, N4], bf16, tag="Lp")
                for c in range(4):
                    pq = psq.tile([128, 2048], f32, tag="psq")
                    for k in range(4):
                        pemm(out=pq[:, 512 * k:512 * (k + 1)], lhsT=wt[:],
                             rhs=gl[:, N3 * c + 512 * k:N3 * c + 512 * (k + 1)],
                             start=True, stop=True, skip_group_check=True)
                    aact(Lp[:, N3 * c:N3 * (c + 1)], pq[:], b1)

                # ---- raw leaf sums: DVE quarter folds ----
                t0 = wkpool.tile([128, N3], bf16, tag="t0")
                vtt(ADD, t0[:], gq[0], gq[1])
                t1 = wkpool.tile([128, N3], bf16, tag="t1")
                vtt(ADD, t1[:], gq[2], gq[3])
                u = wkpool.tile([128, N3], bf16, tag="u")
                vtt(ADD, u[:], t0[:], t1[:])
                s3raw = wkpool.tile([128, N3], bf16, tag="s3raw")
                vtt(ADD, s3raw[:], u[:], gu[:, U3:U3 + N3])

                # ---- leaf max folds ----
                x0 = wkpool.tile([128, N3], bf16, tag="x0")
                vtt(MAX, x0[:], Lp[:, 0:N3], Lp[:, N3:2 * N3])
                x1 = wkpool.tile([128, N3], bf16, tag="x1")
                vtt(MAX, x1[:], Lp[:, 2 * N3:3 * N3], Lp[:, 3 * N3:4 * N3])
                m4 = wkpool.tile([128, N3], bf16, tag="m4")
                vtt(MAX, m4[:], x0[:], x1[:])

                # ---- s3 projection + 5b drain, combine ----
                p3 = psq.tile([128, 2048], f32, tag="psq")
                for k in range(4):
                    pemm(out=p3[:, 512 * k:512 * (k + 1)], lhsT=wt[:],
                         rhs=s3raw[:, 512 * k:512 * (k + 1)],
                         start=True, stop=True, skip_group_check=True)
                s3p = drpool.tile([128, N3], bf16, tag="s3p")
                aact(s3p[:], p3[:], b5)
                m34 = wkpool.tile([128, N3], bf16, tag="m34")
                vtt(MAX, m34[:], m4[:], s3p[:])

                # ---- cascade L3->root: fold, accum-proj, drain, max ----
                pu = psq.tile([128, 2048], f32, tag="psq")
                pu2 = None
                mprev = m34
                sprev = s3p
                for (nl, uoff, o_lo, o_hi) in (
                    (N2, U2, 0, 512),
                    (N1, U1, 0, 128),
                    (N0, U0, 512, 544),
                ):
                    if nl == N1:
                        # s1p/s0p accumulate in a fresh rotation so the fat
                        # s2p tile releases right after its drain
                        pu2 = psq.tile([128, 2048], f32, tag="psq")
                    if nl != N2:
                        pu = pu2
                    q = nl  # quarter width of the PREVIOUS level block
                    f0 = wkpool.tile([128, nl], bf16, tag=f"f0_{nl}")
                    vtt(MAX, f0[:], mprev[:, 0:q], mprev[:, q:2 * q])
                    f1 = wkpool.tile([128, nl], bf16, tag=f"f1_{nl}")
                    vtt(MAX, f1[:], mprev[:, 2 * q:3 * q], mprev[:, 3 * q:4 * q])
                    ff = wkpool.tile([128, nl], bf16, tag=f"ff_{nl}")
                    vtt(MAX, ff[:], f0[:], f1[:])
                    ps_slice = pu[:, o_lo:o_hi]
                    pemm(out=ps_slice, lhsT=wt[:], rhs=gu[:, uoff:uoff + nl],
                         start=True, stop=False, skip_group_check=True)
                    for c in range(4):
                        pemm(out=ps_slice, lhsT=ident[:],
                             rhs=sprev[:, nl * c:nl * (c + 1)],
                             start=False, stop=(c == 3), skip_group_check=True)
                    sp = drpool.tile([128, nl], bf16, tag=f"sp_{nl}")
                    aact(sp[:], ps_slice, b1)
                    if nl == N0:
                        vtt(MAX, R[:, TPS * s:TPS * (s + 1)], ff[:], sp[:])
                    else:
                        mnew = wkpool.tile([128, nl], bf16, tag=f"m_{nl}")
                        vtt(MAX, mnew[:], ff[:], sp[:])
                        mprev = mnew
                        sprev = sp

            # ---- transpose [d', tree] -> [tree, d'] and store ----
            Rt = cpool.tile([TPC, D], bf16)
            em(nc.sync, lambda: nc.sync.dma_start_transpose(out=Rt[:], in_=R[:]))
            osb = cpool.tile([TPC, D], f32)
            em(nc.scalar, lambda: nc.scalar.copy(out=osb[:], in_=Rt[:]))
            em(nc.sync, lambda: nc.sync.dma_start(out=outd[:], in_=osb[:]))
            # carriers for the kernel-tail drain's global-clock waits
            for _ in range(20):
                nop = nc.sync.nop(nofuse=True)
                if last_on.get(id(nc.sync)) is not None:
                    _adh(nop.ins, last_on[id(nc.sync)], sync=False,
                         reason="drain carrier")
                last_on[id(nc.sync)] = nop.ins
    return nc, name2idx


def _distribute_waits(nc, name2idx):
    """Move excess sync waits (walrus allows one per instruction) onto the
    carrier nops glued before each instruction. Returns {emission_idx:
    carriers_needed} for instructions that still lack carriers."""
    import bass_rust
    missing = {}
    pending = {}     # survives across blocks: layout order is execution order
    for blk in nc.m.functions[0].blocks:
        for inst in blk.instructions:
            eng = getattr(inst, "engine", None)
            if eng is None:
                continue
            key = str(eng)
            ty = type(inst).__name__
            si_ld = inst.sync_info
            if ty == "InstUnconditionalBranch" or (
                ty == "InstLdweights"
                and (si_ld is None or len(si_ld.on_wait) <= 1)
            ):
                continue            # transparent: carriers before these still
                                    # execute (in order) on this engine
            if ty == "InstNoOp":
                pending.setdefault(key, []).append(inst)
                continue
            si = inst.sync_info
            w = [] if si is None else list(si.on_wait)
            if len(w) > 1:
                free = [n for n in pending.get(key, [])
                        if n.sync_info is None or not n.sync_info.on_wait]
                extra = w[1:]
                if inst.name not in name2idx:
                    if ty == "InstEventSemaphore" and len(w) <= 2:
                        pending[key] = []
                        continue
                    if len(extra) <= len(free):
                        for wt_, nop in zip(extra, reversed(free)):
                            nop.sync_info = bass_rust.SyncInfo(
                                on_wait=[wt_], on_update=[])
                        si.on_wait = w[:1]
                        pending[key] = []
                        continue
                    raise AssertionError(
                        f"{inst.name} ({ty}): {len(extra)} excess waits, "
                        f"{len(free)} free carriers, no emission site")
                if len(extra) > len(free):
                    missing[name2idx[inst.name]] = len(extra)
                else:
                    for wt_, nop in zip(extra, reversed(free)):
                        nop.sync_info = bass_rust.SyncInfo(
                            on_wait=[wt_], on_update=[])
                    si.on_wait = w[:1]
            pending[key] = []
    return missing


def _build_nc():
    sites = {}
    missing = {}
    for _ in range(12):
        nc, name2idx = _build_once(sites)
        missing = _distribute_waits(nc, name2idx)
        if not missing:
            for blk in nc.m.functions[0].blocks:
                for inst in blk.instructions:
                    si = inst.sync_info
                    if si is not None and len(si.on_wait) > 1:
                        ty = type(inst).__name__
                        assert ty == "InstEventSemaphore" and len(si.on_wait) <= 2, (
                            f"{inst.name} ({ty}) kept {len(si.on_wait)} waits")
            return nc
        for i, n in missing.items():
            sites[i] = max(sites.get(i, 0), n)
    raise RuntimeError(f"wait-carrier fixpoint did not converge: {missing}")


def _lambda_orders():
    """Quarter-major (tree, local-node) orders per level for one superchunk.
    Lambda_0 = tree order; Lambda_l = [children c of Lambda_{l-1}, c-major]."""
    lam = [[(t, 0) for t in range(TPS)]]
    for _ in range(4):
        prev = lam[-1]
        lam.append([(t, 4 * loc + 1 + c) for c in range(4) for (t, loc) in prev])
    return lam


def _host_inputs(tokens, emb, W, b):
    import ml_dtypes
    bf16 = ml_dtypes.bfloat16

    toks = np.asarray(tokens).reshape(B, NPT)
    embb = np.asarray(emb, dtype=np.float32).astype(bf16)
    lam = _lambda_orders()
    # per-level index arrays [TPS-tree-relative flat indices]
    idx_l = [np.array([t * NPT + loc for (t, loc) in lv]) for lv in lam]
    gxls, gxus = [], []
    for c in range(NCORES):
        tc_ = toks[TPC * c:TPC * (c + 1)]
        lcols, ucols = [], []
        for s in range(SC):
            ts = tc_[TPS * s:TPS * (s + 1)].reshape(-1)   # [32*341] token ids
            lcols.append(ts[idx_l[4]])
            for li in (idx_l[3], idx_l[2], idx_l[1], idx_l[0]):
                ucols.append(ts[li])
        gxls.append(np.ascontiguousarray(embb[np.concatenate(lcols)].T))
        gxus.append(np.ascontiguousarray(embb[np.concatenate(ucols)].T))
    W = np.asarray(W, dtype=np.float32)
    b = np.asarray(b, dtype=np.float32)
    wt = np.ascontiguousarray(W.T).astype(bf16)
    ident = np.eye(D, dtype=np.float32).astype(bf16)
    bcol = np.stack([b, 5.0 * b], axis=1).astype(np.float32)
    return gxls, gxus, wt, ident, bcol


def kernel(tokens, parent, batch_id, emb, W, b, bs, **_):
    from concourse.bass_utils import run_bass_kernel_spmd

    if "nc" not in _compiled:
        _compiled["nc"] = _build_nc()
    nc = _compiled["nc"]

    gxls, gxus, wt, ident, bcol = _host_inputs(tokens, emb, W, b)
    in_maps = [
        {"gxl": gxls[c], "gxu": gxus[c], "wt": wt, "ident": ident, "bcol": bcol}
        for c in range(NCORES)
    ]
    res = run_bass_kernel_spmd(nc, in_maps, list(range(NCORES)))
    out = np.concatenate([res.results[c]["out"] for c in range(NCORES)], axis=0)
    return out.astype(np.float32)


# revision 20
# speedup vs baseline: 1.0170x; 1.0170x over previous
"""BatchTreeEncoder Trainium2 kernel.

Forest of B=1024 identical complete 4-ary trees (341 nodes, 5 levels).
reference: e_v = W emb[tok_v] + b; s_v = subtree sum of e; out = per-tree
elementwise max over all s_v.

Strategy (data-parallel over trees, 128 trees/core on 8 cores, all
engines balanced):
  * Host gathers embeddings bf16 and TRANSPOSED ([128 channels, node
    cols]) in quarter-major (Lambda) order: each level's superchunk
    block is [q0|q1|q2|q3] where q_c holds the c-th children of the
    parent level's own Lambda order (Lambda_0 = tree order).  Every
    4-way child fold -- sum or max, at every level -- is then three
    tensor_tensor ops on CONTIGUOUS slices (guaranteed 2x DVE mode).
  * PE projects leaves and s3 sums with stationary W^T (N=512 streams,
    no transposes, no LDWEIGHTS thrash), and builds upper-level
    projections by PSUM-accumulating identity matmuls over the drained
    child projections on top of W^T @ e_raw.
  * ACT drains every projection PSUM->SBUF bf16 with the node bias
    fused (+b, +5b for s3); the per-drain +b telescopes to exactly
    |subtree|*b for every node value.
  * DVE folds raw leaf sums, the leaf max quartet, and the max cascade
    down the levels; the last fold writes the per-tree answer directly.
  * Final [d,tree]->[tree,d] via HWDGE DMA transpose.

The installed walrus gives every engine instruction a single sync-wait
slot, so _build_nc runs a fixpoint: build, find instructions that were
assigned >1 wait, rebuild with carrier nops (one wait each) glued
immediately before those instructions on the same engine.
"""

import sys

sys.path.insert(0, "/opt/trn_rl_repo")

import numpy as np

B = 1024
NPT = 341
VOCAB = 50000
D = 128
NCORES = 8
TPC = B // NCORES          # 128 trees per core
SC = 4                     # superchunks per core
TPS = TPC // SC            # 32 trees per superchunk

N4 = TPS * 256             # 8192 leaf cols per superchunk
N3 = TPS * 64              # 2048
N2 = TPS * 16              # 512
N1 = TPS * 4               # 128
N0 = TPS * 1               # 32
UW = N3 + N2 + N1 + N0     # 2720 upper cols per superchunk
U3, U2, U1, U0 = 0, N3, N3 + N2, N3 + N2 + N1

_compiled = {}


def _build_once(sites):
    """Build the kernel; emission index i gets sites.get(i, 0) carrier nops
    glued immediately before it on its engine. Returns (nc, name2idx)."""
    import concourse.bass as bass
    import concourse.mybir as mybir
    import concourse.tile as tile
    from bass_rust import add_dep_helper as _adh

    f32 = mybir.dt.float32
    bf16 = mybir.dt.bfloat16
    T = mybir.ActivationFunctionType
    MAX = mybir.AluOpType.max
    ADD = mybir.AluOpType.add

    nc = bass.Bass()
    gxld = nc.declare_dram_parameter("gxl", [128, SC * N4], bf16, isOutput=False)
    gxud = nc.declare_dram_parameter("gxu", [128, SC * UW], bf16, isOutput=False)
    wtd = nc.declare_dram_parameter("wt", [D, D], bf16, isOutput=False)
    idd = nc.declare_dram_parameter("ident", [D, D], bf16, isOutput=False)
    bcd = nc.declare_dram_parameter("bcol", [D, 2], f32, isOutput=False)   # [b, 5b]
    outd = nc.declare_dram_parameter("out", [TPC, D], f32, isOutput=True)

    emidx = [0]
    name2idx = {}
    last_on = {}

    def em(eng, maker):
        # emission wrapper: chains each engine's instructions in emission
        # order (nosync deps only) so carrier nops stay adjacent to the
        # instruction whose excess waits they will carry
        i = emidx[0]
        emidx[0] += 1
        for _ in range(sites.get(i, 0)):
            nop = eng.nop(nofuse=True)
            if last_on.get(id(eng)) is not None:
                _adh(nop.ins, last_on[id(eng)], sync=False, reason="carrier order")
            last_on[id(eng)] = nop.ins
        inst = maker()
        if last_on.get(id(eng)) is not None:
            _adh(inst.ins, last_on[id(eng)], sync=False, reason="carrier order")
        last_on[id(eng)] = inst.ins
        name2idx[inst.ins.name] = i
        return inst

    with tile.TileContext(nc) as tc:
        with (
            tc.tile_pool(name="const", bufs=1) as cpool,
            tc.tile_pool(name="gq", bufs=3) as gqpool,
            tc.tile_pool(name="gu", bufs=3) as gupool,
            tc.tile_pool(name="lp", bufs=3) as lppool,
            tc.tile_pool(name="wk", bufs=2) as wkpool,
            tc.tile_pool(name="dr", bufs=2) as drpool,
            tc.tile_pool(name="psq", bufs=2, space="PSUM") as psq,
        ):
            def pemm(**kw):
                return em(nc.tensor, lambda: nc.tensor.matmul(**kw))

            def aact(out, in_, bias):
                return em(nc.scalar, lambda: nc.scalar.activation(
                    out=out, in_=in_, func=T.Identity, bias=bias, scale=1.0))

            def vtt(op, out, in0, in1):
                return em(nc.vector, lambda: nc.vector.tensor_tensor(
                    out=out, in0=in0, in1=in1, op=op))

            wt = cpool.tile([D, D], bf16)
            em(nc.sync, lambda: nc.sync.dma_start(out=wt[:], in_=wtd[:]))
            ident = cpool.tile([D, D], bf16)
            em(nc.sync, lambda: nc.sync.dma_start(out=ident[:], in_=idd[:]))
            bcol = cpool.tile([D, 2], f32)
            em(nc.sync, lambda: nc.sync.dma_start(out=bcol[:], in_=bcd[:]))
            b1 = bcol[:, 0:1]
            b5 = bcol[:, 1:2]

            R = cpool.tile([D, TPC], bf16)     # per-tree answers, [d', tree]

            for s in range(SC):
                gl = gqpool.tile([128, N4], bf16, tag="gl")
                em(nc.sync, lambda gl=gl, s=s: nc.sync.dma_start(
                    out=gl[:], in_=gxld[:, N4 * s:N4 * (s + 1)]))
                gq = [gl[:, N3 * c:N3 * (c + 1)] for c in range(4)]
                gu = gupool.tile([128, UW], bf16, tag="gu")
                em(nc.sync, lambda gu=gu, s=s: nc.sync.dma_start(
                    out=gu[:], in_=gxud[:, UW * s:UW * (s + 1)]))

                # ---- leaf projections + biased drains (node values) ----
                Lp = lppool.tile([128, N4], bf16, tag="Lp")
                for c in range(4):
                    pq = psq.tile([128, 2048], f32, tag="psq")
                    for k in range(4):
                        pemm(out=pq[:, 512 * k:512 * (k + 1)], lhsT=wt[:],
                             rhs=gl[:, N3 * c + 512 * k:N3 * c + 512 * (k + 1)],
                             start=True, stop=True, skip_group_check=True)
                    aact(Lp[:, N3 * c:N3 * (c + 1)], pq[:], b1)

                # ---- raw leaf sums: DVE quarter folds ----
                t0 = wkpool.tile([128, N3], bf16, tag="t0")
                vtt(ADD, t0[:], gq[0], gq[1])
                t1 = wkpool.tile([128, N3], bf16, tag="t1")
                vtt(ADD, t1[:], gq[2], gq[3])
                u = wkpool.tile([128, N3], bf16, tag="u")
                vtt(ADD, u[:], t0[:], t1[:])
                s3raw = wkpool.tile([128, N3], bf16, tag="s3raw")
                vtt(ADD, s3raw[:], u[:], gu[:, U3:U3 + N3])

                # ---- leaf max folds ----
                x0 = wkpool.tile([128, N3], bf16, tag="x0")
                vtt(MAX, x0[:], Lp[:, 0:N3], Lp[:, N3:2 * N3])
                x1 = wkpool.tile([128, N3], bf16, tag="x1")
                vtt(MAX, x1[:], Lp[:, 2 * N3:3 * N3], Lp[:, 3 * N3:4 * N3])
                m4 = wkpool.tile([128, N3], bf16, tag="m4")
                vtt(MAX, m4[:], x0[:], x1[:])

                # ---- s3 projection + 5b drain, combine ----
                p3 = psq.tile([128, 2048], f32, tag="psq")
                for k in range(4):
                    pemm(out=p3[:, 512 * k:512 * (k + 1)], lhsT=wt[:],
                         rhs=s3raw[:, 512 * k:512 * (k + 1)],
                         start=True, stop=True, skip_group_check=True)
                s3p = drpool.tile([128, N3], bf16, tag="s3p")
                aact(s3p[:], p3[:], b5)
                m34 = wkpool.tile([128, N3], bf16, tag="m34")
                vtt(MAX, m34[:], m4[:], s3p[:])

                # ---- cascade L3->root: fold, accum-proj, drain, max ----
                pu = psq.tile([128, 2048], f32, tag="psq")
                mprev = m34
                sprev = s3p
                for (nl, uoff, o_lo, o_hi) in (
                    (N2, U2, 0, 512),
                    (N1, U1, 512, 640),
                    (N0, U0, 1024, 1056),
                ):
                    q = nl  # quarter width of the PREVIOUS level block
                    f0 = wkpool.tile([128, nl], bf16, tag=f"f0_{nl}")
                    vtt(MAX, f0[:], mprev[:, 0:q], mprev[:, q:2 * q])
                    f1 = wkpool.tile([128, nl], bf16, tag=f"f1_{nl}")
                    vtt(MAX, f1[:], mprev[:, 2 * q:3 * q], mprev[:, 3 * q:4 * q])
                    ff = wkpool.tile([128, nl], bf16, tag=f"ff_{nl}")
                    vtt(MAX, ff[:], f0[:], f1[:])
                    ps_slice = pu[:, o_lo:o_hi]
                    pemm(out=ps_slice, lhsT=wt[:], rhs=gu[:, uoff:uoff + nl],
                         start=True, stop=False, skip_group_check=True)
                    for c in range(4):
                        pemm(out=ps_slice, lhsT=ident[:],
                             rhs=sprev[:, nl * c:nl * (c + 1)],
                             start=False, stop=(c == 3), skip_group_check=True)
                    sp = drpool.tile([128, nl], bf16, tag=f"sp_{nl}")
                    aact(sp[:], ps_slice, b1)
                    if nl == N0:
                        vtt(MAX, R[:, TPS * s:TPS * (s + 1)], ff[:], sp[:])
                    else:
                        mnew = wkpool.tile([128, nl], bf16, tag=f"m_{nl}")
                        vtt(MAX, mnew[:], ff[:], sp[:])
                        mprev = mnew
                        sprev = sp

            # ---- transpose [d', tree] -> [tree, d'] and store ----
            Rt = cpool.tile([TPC, D], bf16)
            em(nc.sync, lambda: nc.sync.dma_start_transpose(out=Rt[:], in_=R[:]))
            osb = cpool.tile([TPC, D], f32)
            em(nc.scalar, lambda: nc.scalar.copy(out=osb[:], in_=Rt[:]))
            em(nc.sync, lambda: nc.sync.dma_start(out=outd[:], in_=osb[:]))
            # carriers for the kernel-tail drain's global-clock waits
            for _ in range(20):
                nop = nc.sync.nop(nofuse=True)
                if last_on.get(id(nc.sync)) is not None:
                    _adh(nop.ins, last_on[id(nc.sync)], sync=False,
                         reason="drain carrier")
                last_on[id(nc.sync)] = nop.ins
    return nc, name2idx


def _distribute_waits(nc, name2idx):
    """Move excess sync waits (walrus allows one per instruction) onto the
    carrier nops glued before each instruction. Returns {emission_idx:
    carriers_needed} for instructions that still lack carriers."""
    import bass_rust
    missing = {}
    pending = {}     # survives across blocks: layout order is execution order
    for blk in nc.m.functions[0].blocks:
        for inst in blk.instructions:
            eng = getattr(inst, "engine", None)
            if eng is None:
                continue
            key = str(eng)
            ty = type(inst).__name__
            si_ld = inst.sync_info
            if ty == "InstUnconditionalBranch" or (
                ty == "InstLdweights"
                and (si_ld is None or len(si_ld.on_wait) <= 1)
            ):
                continue            # transparent: carriers before these still
                                    # execute (in order) on this engine
            if ty == "InstNoOp":
                pending.setdefault(key, []).append(inst)
                continue
            si = inst.sync_info
            w = [] if si is None else list(si.on_wait)
            if len(w) > 1:
                free = [n for n in pending.get(key, [])
                        if n.sync_info is None or not n.sync_info.on_wait]
                extra = w[1:]
                if inst.name not in name2idx:
                    if ty == "InstEventSemaphore" and len(w) <= 2:
                        pending[key] = []
                        continue
                    if len(extra) <= len(free):
                        for wt_, nop in zip(extra, reversed(free)):
                            nop.sync_info = bass_rust.SyncInfo(
                                on_wait=[wt_], on_update=[])
                        si.on_wait = w[:1]
                        pending[key] = []
                        continue
                    raise AssertionError(
                        f"{inst.name} ({ty}): {len(extra)} excess waits, "
                        f"{len(free)} free carriers, no emission site")
                if len(extra) > len(free):
                    missing[name2idx[inst.name]] = len(extra)
                else:
                    for wt_, nop in zip(extra, reversed(free)):
                        nop.sync_info = bass_rust.SyncInfo(
                            on_wait=[wt_], on_update=[])
                    si.on_wait = w[:1]
            pending[key] = []
    return missing


def _build_nc():
    sites = {}
    missing = {}
    for _ in range(12):
        nc, name2idx = _build_once(sites)
        missing = _distribute_waits(nc, name2idx)
        if not missing:
            for blk in nc.m.functions[0].blocks:
                for inst in blk.instructions:
                    si = inst.sync_info
                    if si is not None and len(si.on_wait) > 1:
                        ty = type(inst).__name__
                        assert ty == "InstEventSemaphore" and len(si.on_wait) <= 2, (
                            f"{inst.name} ({ty}) kept {len(si.on_wait)} waits")
            return nc
        for i, n in missing.items():
            sites[i] = max(sites.get(i, 0), n)
    raise RuntimeError(f"wait-carrier fixpoint did not converge: {missing}")


def _lambda_orders():
    """Quarter-major (tree, local-node) orders per level for one superchunk.
    Lambda_0 = tree order; Lambda_l = [children c of Lambda_{l-1}, c-major]."""
    lam = [[(t, 0) for t in range(TPS)]]
    for _ in range(4):
        prev = lam[-1]
        lam.append([(t, 4 * loc + 1 + c) for c in range(4) for (t, loc) in prev])
    return lam


def _host_inputs(tokens, emb, W, b):
    import ml_dtypes
    bf16 = ml_dtypes.bfloat16

    toks = np.asarray(tokens).reshape(B, NPT)
    embb = np.asarray(emb, dtype=np.float32).astype(bf16)
    lam = _lambda_orders()
    # per-level index arrays [TPS-tree-relative flat indices]
    idx_l = [np.array([t * NPT + loc for (t, loc) in lv]) for lv in lam]
    gxls, gxus = [], []
    for c in range(NCORES):
        tc_ = toks[TPC * c:TPC * (c + 1)]
        lcols, ucols = [], []
        for s in range(SC):
            ts = tc_[TPS * s:TPS * (s + 1)].reshape(-1)   # [32*341] token ids
            lcols.append(ts[idx_l[4]])
            for li in (idx_l[3], idx_l[2], idx_l[1], idx_l[0]):
                ucols.append(ts[li])
        gxls.append(np.ascontiguousarray(embb[np.concatenate(lcols)].T))
        gxus.append(np.ascontiguousarray(embb[np.concatenate(ucols)].T))
    W = np.asarray(W, dtype=np.float32)
    b = np.asarray(b, dtype=np.float32)
    wt = np.ascontiguousarray(W.T).astype(bf16)
    ident = np.eye(D, dtype=np.float32).astype(bf16)
    bcol = np.stack([b, 5.0 * b], axis=1).astype(np.float32)
    return gxls, gxus, wt, ident, bcol


def kernel(tokens, parent, batch_id, emb, W, b, bs, **_):
    from concourse.bass_utils import run_bass_kernel_spmd

    if "nc" not in _compiled:
        _compiled["nc"] = _build_nc()
    nc = _compiled["nc"]

    gxls, gxus, wt, ident, bcol = _host_inputs(tokens, emb, W, b)
    in_maps = [
        {"gxl": gxls[c], "gxu": gxus[c], "wt": wt, "ident": ident, "bcol": bcol}
        for c in range(NCORES)
    ]
    res = run_bass_kernel_spmd(nc, in_maps, list(range(NCORES)))
    out = np.concatenate([res.results[c]["out"] for c in range(NCORES)], axis=0)
    return out.astype(np.float32)
